# revision 4
# baseline (speedup 1.0000x reference)
"""Self-contained Bass/Trainium2 kernel for the 2-layer LSTM + linear head.

Problem: x [2048, 512, 8] -> 2-layer LSTM (H=50, PyTorch gate order i,f,g,o)
-> last hidden state of layer 2 -> linear [1, 50] -> y [2048, 1].

Strategy: pure data parallel over 8 NeuronCores (256 batch rows each). On
each core the batch is further split into two independent 128-wide
sub-batch pipelines so the serial T=512 recurrence latency is overlapped.

Per-core layout (per sub-batch sb, 128 batch columns on the free dim):
  - Gate rows on partitions, padded to 64-row blocks so every engine access
    starts at a 32-aligned partition base: chunk A = [i rows 0:50 | f rows
    64:114], chunk B = [g rows 0:50 | o rows 64:114]. The g block's weights
    are pre-scaled by 2 so tanh(z) = 2*sigmoid(2z) - 1 needs only sigmoid.
  - One rhs tile R [128, 128] per step: rows 0:50 h0, 50:58 x_t, 58 ones
    (bias row), 59:64 zeros, 64:114 h1 (layer 2 runs one step behind layer
    1 - the skew makes both layers' gate matmuls computable in the same
    iteration). Biases are folded into the matmul via the ones row.
  - 4 matmuls per sb per step into one PSUM tile g [128, 512] (cols
    A-L0 | A-L1 | B-L0 | B-L1), one sigmoid over all gates, then the cell
    update on VectorE: u' = 2*i*sig2g (fused scalar_tensor_tensor),
    t1 = u' - i  (so t1 = i*tanh(zg)), v = f*c, c' = t1 + v, tanh(c') on
    ScalarE, h0/h1 products written straight into the next step's rhs tile.
"""
import numpy as np
import concourse.bacc as bacc
import concourse.mybir as mybir
from concourse.tile import TileContext
from concourse.bass_utils import run_bass_kernel_spmd

f32 = mybir.dt.float32
AF = mybir.ActivationFunctionType
ALU = mybir.AluOpType

H = 50
D = 8
B = 2048
T = 512
NCORES = 8
BC = B // NCORES   # 256 batch rows per core
NSB = 2
SB = BC // NSB     # 128 batch cols per sub-batch

_NC_CACHE = {}


def _build_nc(repeat=1):
    nc = bacc.Bacc(None, target_bir_lowering=False)

    xT = nc.dram_tensor("xT", [T, 14, BC], f32, kind="ExternalInput")
    w0a = nc.dram_tensor("w0a", [59, 128], f32, kind="ExternalInput")
    w0b = nc.dram_tensor("w0b", [59, 128], f32, kind="ExternalInput")
    w1a = nc.dram_tensor("w1a", [115, 128], f32, kind="ExternalInput")
    w1b = nc.dram_tensor("w1b", [115, 128], f32, kind="ExternalInput")
    wfin = nc.dram_tensor("wfin", [128, 1], f32, kind="ExternalInput")
    y = nc.dram_tensor("y", [1, BC], f32, kind="ExternalOutput")

    with TileContext(nc) as tc:
        with (
            tc.tile_pool(name="wp", bufs=1) as wp,
            tc.tile_pool(name="st", bufs=1) as st,
            tc.tile_pool(name="rp", bufs=3) as rp,
            tc.tile_pool(name="sp", bufs=2) as sp,
            tc.tile_pool(name="tp", bufs=2) as tp,
            tc.tile_pool(name="gp", bufs=2, space="PSUM") as gp,
        ):
            W0A = wp.tile([59, 128], f32, name="W0A")
            W0B = wp.tile([59, 128], f32, name="W0B")
            W1A = wp.tile([115, 128], f32, name="W1A")
            W1B = wp.tile([115, 128], f32, name="W1B")
            WF = wp.tile([128, 1], f32, name="WF")
            nc.sync.dma_start(out=W0A, in_=w0a[:, :])
            nc.sync.dma_start(out=W0B, in_=w0b[:, :])
            nc.sync.dma_start(out=W1A, in_=w1a[:, :])
            nc.sync.dma_start(out=W1B, in_=w1b[:, :])
            nc.sync.dma_start(out=WF, in_=wfin[:, :])

            C = [st.tile([128, 256], f32, name=f"C{sb}") for sb in range(NSB)]
            TH = [st.tile([128, 256], f32, name=f"TH{sb}") for sb in range(NSB)]

            def new_r(sb, t, memset):
                r = rp.tile([128, SB], f32, name=f"rt{sb}", tag=f"r_{sb}")
                if memset:
                    nc.vector.memset(r, 0.0)
                nc.sync.dma_start(out=r[50:64, :],
                                  in_=xT[min(t, T - 1)][:, sb * SB:(sb + 1) * SB])
                return r

            for rep in range(repeat):
              for sb in range(NSB):
                nc.vector.memset(C[sb], 0.0)
              rcur = [new_r(sb, 0, True) for sb in range(NSB)]

              for t in range(T + 1):
                rnext = [new_r(sb, t + 1, t + 1 <= 2) for sb in range(NSB)]
                g = [gp.tile([128, 512], f32, name=f"g{sb}", tag=f"g{sb}")
                     for sb in range(NSB)]
                for sb in range(NSB):
                    nc.tensor.matmul(g[sb][:, 0:128], W0A[0:59, :],
                                     rcur[sb][0:59, :], start=True, stop=True)
                for sb in range(NSB):
                    nc.tensor.matmul(g[sb][:, 256:384], W0B[0:59, :],
                                     rcur[sb][0:59, :], start=True, stop=True)
                for sb in range(NSB):
                    nc.tensor.matmul(g[sb][:, 128:256], W1A[0:114, :],
                                     rcur[sb][0:114, :], start=True, stop=True)
                for sb in range(NSB):
                    nc.tensor.matmul(g[sb][:, 384:512], W1B[0:114, :],
                                     rcur[sb][0:114, :], start=True, stop=True)

                for sb in range(NSB):
                    s = sp.tile([128, 512], f32, name=f"s{sb}", tag=f"s{sb}")
                    nc.scalar.activation(out=s, in_=g[sb][:, :], func=AF.Sigmoid)

                    up = tp.tile([128, 256], f32, name=f"up{sb}", tag=f"up{sb}")
                    t1 = tp.tile([128, 256], f32, name=f"t1{sb}", tag=f"t1{sb}")
                    v = tp.tile([128, 256], f32, name=f"v{sb}", tag=f"v{sb}")
                    # u' = (sig_2g * 2) * i
                    nc.vector.scalar_tensor_tensor(out=up[0:64, :],
                                                   in0=s[0:64, 256:512],
                                                   scalar=2.0, in1=s[0:64, 0:256],
                                                   op0=ALU.mult, op1=ALU.mult)
                    # t1 = u' - i = i * tanh(zg)
                    nc.vector.tensor_tensor(out=t1[64:128, :], in0=up[0:64, :],
                                            in1=s[0:64, 0:256], op=ALU.subtract)
                    # v = f * c
                    nc.vector.tensor_tensor(out=v[64:128, :], in0=s[64:128, 0:256],
                                            in1=C[sb][64:128, :], op=ALU.mult)
                    # c' = t1 + v
                    nc.vector.tensor_tensor(out=C[sb][64:128, :], in0=t1[64:128, :],
                                            in1=v[64:128, :], op=ALU.add)
                    # th = tanh(c')
                    nc.scalar.activation(out=TH[sb][64:128, :], in_=C[sb][64:128, :],
                                         func=AF.Tanh)
                    # h = o * th; layer-1 half feeds rows 0:50, layer-2 rows 64:114
                    nc.vector.tensor_tensor(out=rnext[sb][0:50, :],
                                            in0=s[64:114, 256:384],
                                            in1=TH[sb][64:114, 0:128], op=ALU.mult)
                    nc.vector.tensor_tensor(out=rnext[sb][64:114, :],
                                            in0=s[64:114, 384:512],
                                            in1=TH[sb][64:114, 128:256], op=ALU.mult)

                if t == 0:
                    # layer 2 ran on junk at t=0 (its real step 0 happens at t=1)
                    for sb in range(NSB):
                        nc.vector.memset(C[sb][64:128, 128:256], 0.0)
                        nc.vector.memset(rnext[sb][64:114, :], 0.0)
                rcur = rnext

            ysb = st.tile([1, BC], f32, name="ysb")
            for sb in range(NSB):
                fin = gp.tile([1, SB], f32, name=f"fin{sb}", tag=f"g{sb}")
                nc.tensor.matmul(fin[:, :], WF[64:114, :], rcur[sb][64:114, :],
                                 start=True, stop=True)
                nc.scalar.copy(out=ysb[:, sb * SB:(sb + 1) * SB], in_=fin[:, :])
            nc.sync.dma_start(out=y[:, :], in_=ysb)

    nc.compile()
    return nc


def _prep_weights(Wih0, Whh0, bih0, bhh0, Wih1, Whh1, bih1, bhh1):
    """Stacked/padded lhsT blobs; biases in K-row 58 (the rhs ones row)."""
    b0 = (np.asarray(bih0) + np.asarray(bhh0)).astype(np.float32)
    b1 = (np.asarray(bih1) + np.asarray(bhh1)).astype(np.float32)

    def chunk(hrows, xrows, onerow, Wh, Wx, b, g0, g1, krows, sc0=1.0, sc1=1.0):
        out = np.zeros((krows, 128), dtype=np.float32)
        for col0, gi, sc in ((0, g0, sc0), (64, g1, sc1)):
            rows = slice(gi * H, (gi + 1) * H)
            out[hrows, col0:col0 + H] = np.asarray(Wh)[rows, :].T * sc
            out[xrows, col0:col0 + H] = np.asarray(Wx)[rows, :].T * sc
            out[onerow, col0:col0 + H] = b[rows] * sc
        return out

    w0a = chunk(slice(0, 50), slice(50, 58), 58, Whh0, Wih0, b0, 0, 1, 59)
    w0b = chunk(slice(0, 50), slice(50, 58), 58, Whh0, Wih0, b0, 2, 3, 59, 2.0, 1.0)
    w1a = chunk(slice(64, 114), slice(0, 50), 58, Whh1, Wih1, b1, 0, 1, 115)
    w1b = chunk(slice(64, 114), slice(0, 50), 58, Whh1, Wih1, b1, 2, 3, 115, 2.0, 1.0)
    return w0a, w0b, w1a, w1b


def make_in_maps(inputs):
    x = np.asarray(inputs["x"], dtype=np.float32)
    w0a, w0b, w1a, w1b = _prep_weights(
        inputs["Wih0"], inputs["Whh0"], inputs["bih0"], inputs["bhh0"],
        inputs["Wih1"], inputs["Whh1"], inputs["bih1"], inputs["bhh1"])
    wfin = np.zeros((128, 1), np.float32)
    wfin[64:114, 0] = np.asarray(inputs["Wlin"], dtype=np.float32)[0, :]

    in_maps = []
    for c in range(NCORES):
        xc = x[c * BC:(c + 1) * BC]              # [BC, T, D]
        xt = np.zeros((T, 14, BC), dtype=np.float32)
        xt[:, 0:D, :] = xc.transpose(1, 2, 0)
        xt[:, D, :] = 1.0                        # ones row (bias)
        in_maps.append({"xT": xt, "w0a": w0a, "w0b": w0b, "w1a": w1a,
                        "w1b": w1b, "wfin": wfin})
    return in_maps


def kernel(x, Wih0, Whh0, bih0, bhh0, Wih1, Whh1, bih1, bhh1, Wlin, blin):
    if "nc" not in _NC_CACHE:
        _NC_CACHE["nc"] = _build_nc()
    nc = _NC_CACHE["nc"]

    in_maps = make_in_maps(dict(x=x, Wih0=Wih0, Whh0=Whh0, bih0=bih0,
                                bhh0=bhh0, Wih1=Wih1, Whh1=Whh1, bih1=bih1,
                                bhh1=bhh1, Wlin=Wlin, blin=blin))
    res = run_bass_kernel_spmd(nc, in_maps, core_ids=list(range(NCORES)))
    out = np.empty((B, 1), dtype=np.float32)
    blin_v = np.float32(np.asarray(blin).reshape(-1)[0])
    for c in range(NCORES):
        out[c * BC:(c + 1) * BC, 0] = res.results[c]["y"][0] + blin_v
    return out



# revision 5
# speedup vs baseline: 2.8731x; 2.8731x over previous
"""Self-contained Bass/Trainium2 kernel for the 2-layer LSTM + linear head.

Problem: x [2048, 512, 8] -> 2-layer LSTM (H=50, PyTorch gate order i,f,g,o)
-> last hidden state of layer 2 -> linear [1, 50] -> y [2048, 1].

Strategy: pure data parallel over 8 NeuronCores (256 batch rows each), with
each core's batch split into two independent 128-wide sub-batch pipelines so
the serial T=512 recurrence latency is overlapped.

Per-core layout (per sub-batch sb, 128 batch columns on the free dim), all
SBUF data fp16 (matmuls run 4x faster than fp32, DVE elementwise ops hit the
2-byte fast modes; PSUM accumulation stays fp32):

  - Gate-major PSUM tile g [128, 512] per step: col blocks g|i|f|o of 128
    batch cols each; within each block layer-0 occupies partitions 0:50 and
    layer-1 64:114 (layer 1 runs one step behind layer 0 - the skew makes
    both layers' gate matmuls computable in the same iteration).
  - ONE matmul per gate block [M=114, N=128]: the lhsT weight blob holds
    [L0 rows | 14 zero cols | L1 rows], so 4 matmuls/step/sb total.
  - rhs tile r [128, 128] per step: rows 0:50 h0, 64:114 h1, 114:122 x_t,
    122 ones (bias row; biases folded into the matmul via it). K=123.
  - g-gate weights are pre-scaled by 2 so tanh(z) = 2*sigmoid(2z) - 1 needs
    only the shared sigmoid: ONE sigmoid over all four blocks [128, 512],
    then w = 2*s_g - 1 on DVE (tensor_scalar).
  - Cell update on DVE as [128p, 128c] ops covering both layers at once:
    t1 = i*w, v = f*c, c' = t1 + v; tanh(c') on ScalarE; one fused h-write
    h = o*th over partitions 0:114 straight into the next step's rhs tile
    (junk rows 50:64 hit zero weight rows, so they are harmless).
"""
import numpy as np
import concourse.bacc as bacc
import concourse.mybir as mybir
from concourse.tile import TileContext
from concourse.bass_utils import run_bass_kernel_spmd

f32 = mybir.dt.float32
f16 = mybir.dt.float16
AF = mybir.ActivationFunctionType
ALU = mybir.AluOpType

H = 50
D = 8
B = 2048
T = 512
NCORES = 8
BC = B // NCORES   # 256 batch rows per core
NSB = 2
SB = BC // NSB     # 128 batch cols per sub-batch

# psum col offsets per gate block (g,i first: they gate the DVE chain)
COL = {"g": 0, "i": 128, "f": 256, "o": 384}
# weight blob: one 128-col block per gate, cols 0:50 = L0 rows, 64:114 = L1
GBLK = {"g": 0, "i": 1, "f": 2, "o": 3}

_NC_CACHE = {}


def _build_nc(repeat=1):
    nc = bacc.Bacc(None, target_bir_lowering=False)

    xT = nc.dram_tensor("xT", [T, 14, BC], f16, kind="ExternalInput")
    wall = nc.dram_tensor("wall", [123, 512], f16, kind="ExternalInput")
    wfin = nc.dram_tensor("wfin", [128, 1], f16, kind="ExternalInput")
    y = nc.dram_tensor("y", [1, BC], f32, kind="ExternalOutput")

    with TileContext(nc) as tc:
        with (
            tc.tile_pool(name="wp", bufs=1) as wp,
            tc.tile_pool(name="st", bufs=1) as st,
            tc.tile_pool(name="rp", bufs=3) as rp,
            tc.tile_pool(name="sp", bufs=2) as sp,
            tc.tile_pool(name="tp", bufs=2) as tp,
            tc.tile_pool(name="gp", bufs=2, space="PSUM") as gp,
        ):
            WALL = wp.tile([123, 512], f16, name="WALL")
            WF = wp.tile([128, 1], f16, name="WF")
            nc.sync.dma_start(out=WALL, in_=wall[:, :])
            nc.sync.dma_start(out=WF, in_=wfin[:, :])

            C = [st.tile([128, SB], f16, name=f"C{sb}") for sb in range(NSB)]
            TH = [st.tile([128, SB], f16, name=f"TH{sb}") for sb in range(NSB)]

            def new_r(sb, t, memset):
                r = rp.tile([128, SB], f16, name=f"rt{sb}", tag=f"r_{sb}")
                if memset:
                    nc.vector.memset(r, 0.0)
                nc.sync.dma_start(out=r[114:128, :],
                                  in_=xT[min(t, T - 1)][:, sb * SB:(sb + 1) * SB])
                return r

            def emit_mms(sb, rcur_sb):
                g = gp.tile([128, 512], f32, name=f"g{sb}", tag=f"g{sb}")
                for gate in ("g", "i", "f", "o"):
                    c0 = COL[gate]
                    b = GBLK[gate] * 128
                    nc.tensor.matmul(g[0:114, c0:c0 + SB],
                                     WALL[0:123, b:b + 114],
                                     rcur_sb[0:123, :], start=True, stop=True)
                return g

            def emit_act_cell(sb, t, g, rnext_sb):
                s = sp.tile([128, 512], f16, name=f"s{sb}", tag=f"s{sb}")
                nc.scalar.activation(out=s, in_=g[:, :], func=AF.Sigmoid)
                w = tp.tile([128, SB], f16, name=f"w{sb}", tag=f"w{sb}")
                nc.vector.tensor_scalar(out=w, in0=s[:, 0:128],
                                        scalar1=2.0, scalar2=1.0,
                                        op0=ALU.mult, op1=ALU.subtract)
                v = tp.tile([128, SB], f16, name=f"v{sb}", tag=f"v{sb}")
                t1 = tp.tile([128, SB], f16, name=f"t1{sb}", tag=f"t1{sb}")
                # t1 = i * tanh(zg)
                nc.vector.tensor_tensor(out=t1, in0=s[:, 128:256],
                                        in1=w, op=ALU.mult)
                # v = f * c
                nc.vector.tensor_tensor(out=v, in0=s[:, 256:384],
                                        in1=C[sb][:, :], op=ALU.mult)
                # c' = t1 + v
                nc.vector.tensor_tensor(out=C[sb][:, :], in0=t1,
                                        in1=v, op=ALU.add)
                # th = tanh(c')
                nc.scalar.activation(out=TH[sb][:, :], in_=C[sb][:, :],
                                     func=AF.Tanh)
                # h0 -> rows 0:50, h1 -> rows 64:114, junk rows 50:64 harmless
                nc.vector.tensor_tensor(out=rnext_sb[0:114, :],
                                        in0=s[0:114, 384:512],
                                        in1=TH[sb][0:114, :], op=ALU.mult)
                if t == 0:
                    # layer 1 ran on junk at t=0 (its real step 0 is at t=1)
                    nc.vector.memset(C[sb][64:128, :], 0.0)
                    nc.vector.memset(rnext_sb[64:114, :], 0.0)

            for rep in range(repeat):
              for sb in range(NSB):
                nc.vector.memset(C[sb], 0.0)
              rcur = [new_r(sb, 0, True) for sb in range(NSB)]

              for t in range(T + 1):
                rnext = [new_r(sb, t + 1, t + 1 <= 2) for sb in range(NSB)]
                gs = [emit_mms(sb, rcur[sb]) for sb in range(NSB)]
                for sb in range(NSB):
                    emit_act_cell(sb, t, gs[sb], rnext[sb])
                rcur = rnext

            ysb = st.tile([1, BC], f32, name="ysb")
            for sb in range(NSB):
                fin = gp.tile([1, SB], f32, name=f"fin{sb}", tag=f"g{sb}")
                nc.tensor.matmul(fin[:, :], WF[64:114, :], rcur[sb][64:114, :],
                                 start=True, stop=True)
                nc.scalar.copy(out=ysb[:, sb * SB:(sb + 1) * SB], in_=fin[:, :])
            nc.sync.dma_start(out=y[:, :], in_=ysb)

    nc.compile()
    return nc


def _prep_weights(inputs):
    b0 = (np.asarray(inputs["bih0"]) + np.asarray(inputs["bhh0"])).astype(np.float32)
    b1 = (np.asarray(inputs["bih1"]) + np.asarray(inputs["bhh1"])).astype(np.float32)
    Wih0 = np.asarray(inputs["Wih0"], np.float32)
    Whh0 = np.asarray(inputs["Whh0"], np.float32)
    Wih1 = np.asarray(inputs["Wih1"], np.float32)
    Whh1 = np.asarray(inputs["Whh1"], np.float32)

    GI = {"i": 0, "f": 1, "g": 2, "o": 3}
    wall = np.zeros((123, 512), np.float32)
    for gate, gi in GI.items():
        rows = slice(gi * H, (gi + 1) * H)
        sc = 2.0 if gate == "g" else 1.0
        c = GBLK[gate] * 128
        # L0 -> output rows 0:50: recurrent h0 (K 0:50), x (K 114:122), bias
        wall[0:50, c:c + 50] = Whh0[rows, :].T * sc
        wall[114:122, c:c + 50] = Wih0[rows, :].T * sc
        wall[122, c:c + 50] = b0[rows] * sc
        # L1 -> output rows 64:114: input h0 (K 0:50), recurrent h1, bias
        wall[0:50, c + 64:c + 114] = Wih1[rows, :].T * sc
        wall[64:114, c + 64:c + 114] = Whh1[rows, :].T * sc
        wall[122, c + 64:c + 114] = b1[rows] * sc
    return wall.astype(np.float16)


def make_in_maps(inputs):
    x = np.asarray(inputs["x"], dtype=np.float32)
    wall = _prep_weights(inputs)
    wfin = np.zeros((128, 1), np.float16)
    wfin[64:114, 0] = np.asarray(inputs["Wlin"], dtype=np.float32)[0, :]

    in_maps = []
    for c in range(NCORES):
        xc = x[c * BC:(c + 1) * BC]              # [BC, T, D]
        xt = np.zeros((T, 14, BC), dtype=np.float16)
        xt[:, 0:D, :] = xc.transpose(1, 2, 0)
        xt[:, D, :] = 1.0                        # ones row (bias)
        in_maps.append({"xT": xt, "wall": wall, "wfin": wfin})
    return in_maps


def kernel(x, Wih0, Whh0, bih0, bhh0, Wih1, Whh1, bih1, bhh1, Wlin, blin):
    if "nc" not in _NC_CACHE:
        _NC_CACHE["nc"] = _build_nc()
    nc = _NC_CACHE["nc"]

    in_maps = make_in_maps(dict(x=x, Wih0=Wih0, Whh0=Whh0, bih0=bih0,
                                bhh0=bhh0, Wih1=Wih1, Whh1=Whh1, bih1=bih1,
                                bhh1=bhh1, Wlin=Wlin, blin=blin))
    res = run_bass_kernel_spmd(nc, in_maps, core_ids=list(range(NCORES)))
    out = np.empty((B, 1), dtype=np.float32)
    blin_v = np.float32(np.asarray(blin).reshape(-1)[0])
    for c in range(NCORES):
        out[c * BC:(c + 1) * BC, 0] = res.results[c]["y"][0] + blin_v
    return out


# revision 6
# speedup vs baseline: 4.1221x; 1.4348x over previous
"""Self-contained Bass/Trainium2 kernel for the 2-layer LSTM + linear head.

Problem: x [2048, 512, 8] -> 2-layer LSTM (H=50, PyTorch gate order i,f,g,o)
-> last hidden state of layer 2 -> linear [1, 50] -> y [2048, 1].

Strategy: pure data parallel over 8 NeuronCores (256 batch rows each), with
each core's batch split into two independent 128-wide sub-batch pipelines so
the serial T=512 recurrence latency is overlapped.

Per-core layout (per sub-batch sb, 128 batch columns on the free dim), all
SBUF data fp16 (matmuls run 4x faster than fp32, DVE elementwise ops hit the
2-byte fast modes; PSUM accumulation stays fp32):

  - Gate-major PSUM tile g [128, 512] per step: col blocks g|i|f|o of 128
    batch cols each; within each block layer-0 occupies partitions 0:50 and
    layer-1 64:114 (layer 1 runs one step behind layer 0 - the skew makes
    both layers' gate matmuls computable in the same iteration).
  - ONE matmul per gate block [M=114, N=128]: the lhsT weight blob holds
    [L0 rows | 14 zero cols | L1 rows], so 4 matmuls/step/sb total.
  - rhs tile r [128, 128] per step: rows 0:50 h0, 64:114 h1, 114:122 x_t,
    122 ones (bias row; biases folded into the matmul via it). K=123.
  - g-gate weights are pre-scaled by 2 so tanh(z) = 2*sigmoid(2z) - 1 needs
    only the shared sigmoid: ONE sigmoid over all four blocks [128, 512],
    then w = 2*s_g - 1 on DVE (tensor_scalar).
  - Cell update on DVE as [128p, 128c] ops covering both layers at once:
    t1 = i*w, v = f*c, c' = t1 + v; tanh(c') on ScalarE; one fused h-write
    h = o*th over partitions 0:114 straight into the next step's rhs tile
    (junk rows 50:64 hit zero weight rows, so they are harmless).
"""
import numpy as np
import concourse.bacc as bacc
import concourse.mybir as mybir
from concourse.tile import TileContext
from concourse.bass_utils import run_bass_kernel_spmd

f32 = mybir.dt.float32
f16 = mybir.dt.float16
AF = mybir.ActivationFunctionType
ALU = mybir.AluOpType

H = 50
D = 8
B = 2048
T = 512
NCORES = 8
BC = B // NCORES   # 256 batch rows per core
NSB = 2
SB = BC // NSB     # 128 batch cols per sub-batch

# psum col offsets per gate block (g,i first: they gate the DVE chain)
COL = {"g": 0, "i": 128, "f": 256, "o": 384}
# weight blob: one 128-col block per gate, cols 0:50 = L0 rows, 64:114 = L1
GBLK = {"g": 0, "i": 1, "f": 2, "o": 3}

_NC_CACHE = {}


def _build_nc(repeat=1):
    nc = bacc.Bacc(None, target_bir_lowering=False)

    xT = nc.dram_tensor("xT", [T, 14, BC], f16, kind="ExternalInput")
    wall = nc.dram_tensor("wall", [123, 512], f16, kind="ExternalInput")
    wfin = nc.dram_tensor("wfin", [128, 1], f16, kind="ExternalInput")
    y = nc.dram_tensor("y", [1, BC], f32, kind="ExternalOutput")

    with TileContext(nc) as tc:
        with (
            tc.tile_pool(name="wp", bufs=1) as wp,
            tc.tile_pool(name="st", bufs=1) as st,
            tc.tile_pool(name="rp", bufs=3) as rp,
            tc.tile_pool(name="sp", bufs=2) as sp,
            tc.tile_pool(name="tp", bufs=2) as tp,
            tc.tile_pool(name="gp", bufs=2, space="PSUM") as gp,
        ):
            WALL = wp.tile([123, 512], f16, name="WALL")
            WF = wp.tile([128, 1], f16, name="WF")
            nc.sync.dma_start(out=WALL, in_=wall[:, :])
            nc.sync.dma_start(out=WF, in_=wfin[:, :])

            C = [st.tile([128, SB], f16, name=f"C{sb}") for sb in range(NSB)]
            TH = [st.tile([128, SB], f16, name=f"TH{sb}") for sb in range(NSB)]

            def new_r(sb, t, memset):
                r = rp.tile([128, SB], f16, name=f"rt{sb}", tag=f"r_{sb}")
                if memset:
                    nc.vector.memset(r, 0.0)
                nc.sync.dma_start(out=r[114:128, :],
                                  in_=xT[min(t, T - 1)][:, sb * SB:(sb + 1) * SB])
                return r

            def emit_mms(sb, rcur_sb):
                g = gp.tile([128, 512], f32, name=f"g{sb}", tag=f"g{sb}")
                for gate in ("g", "i", "f", "o"):
                    c0 = COL[gate]
                    b = GBLK[gate] * 128
                    nc.tensor.matmul(g[0:114, c0:c0 + SB],
                                     WALL[0:123, b:b + 114],
                                     rcur_sb[0:123, :], start=True, stop=True)
                return g

            def emit_act_cell(sb, t, g, rnext_sb):
                s = sp.tile([128, 512], f16, name=f"s{sb}", tag=f"s{sb}")
                # g|i|f gate the DVE chain; o only feeds the h-write at the
                # end of the cycle, so sigmoid it separately off the chain
                nc.scalar.activation(out=s[:, 0:384], in_=g[:, 0:384],
                                     func=AF.Sigmoid)
                nc.scalar.activation(out=s[:, 384:512], in_=g[:, 384:512],
                                     func=AF.Sigmoid)
                w = tp.tile([128, SB], f16, name=f"w{sb}", tag=f"w{sb}")
                nc.vector.tensor_scalar(out=w, in0=s[:, 0:128],
                                        scalar1=2.0, scalar2=1.0,
                                        op0=ALU.mult, op1=ALU.subtract)
                v = tp.tile([128, SB], f16, name=f"v{sb}", tag=f"v{sb}")
                t1 = tp.tile([128, SB], f16, name=f"t1{sb}", tag=f"t1{sb}")
                # t1 = i * tanh(zg)
                nc.vector.tensor_tensor(out=t1, in0=s[:, 128:256],
                                        in1=w, op=ALU.mult)
                # v = f * c
                nc.vector.tensor_tensor(out=v, in0=s[:, 256:384],
                                        in1=C[sb][:, :], op=ALU.mult)
                # c' = t1 + v
                nc.vector.tensor_tensor(out=C[sb][:, :], in0=t1,
                                        in1=v, op=ALU.add)
                # th = tanh(c')
                nc.scalar.activation(out=TH[sb][:, :], in_=C[sb][:, :],
                                     func=AF.Tanh)
                # h0 -> rows 0:50, h1 -> rows 64:114, junk rows 50:64 harmless
                nc.vector.tensor_tensor(out=rnext_sb[0:114, :],
                                        in0=s[0:114, 384:512],
                                        in1=TH[sb][0:114, :], op=ALU.mult)
                if t == 0:
                    # layer 1 ran on junk at t=0 (its real step 0 is at t=1)
                    nc.vector.memset(C[sb][64:128, :], 0.0)
                    nc.vector.memset(rnext_sb[64:114, :], 0.0)

            for rep in range(repeat):
              for sb in range(NSB):
                nc.vector.memset(C[sb], 0.0)
              rcur = [new_r(sb, 0, True) for sb in range(NSB)]

              for t in range(T + 1):
                rnext = [new_r(sb, t + 1, t + 1 <= 2) for sb in range(NSB)]
                gs = [emit_mms(sb, rcur[sb]) for sb in range(NSB)]
                for sb in range(NSB):
                    emit_act_cell(sb, t, gs[sb], rnext[sb])
                rcur = rnext

            ysb = st.tile([1, BC], f32, name="ysb")
            for sb in range(NSB):
                fin = gp.tile([1, SB], f32, name=f"fin{sb}", tag=f"g{sb}")
                nc.tensor.matmul(fin[:, :], WF[64:114, :], rcur[sb][64:114, :],
                                 start=True, stop=True)
                nc.scalar.copy(out=ysb[:, sb * SB:(sb + 1) * SB], in_=fin[:, :])
            nc.sync.dma_start(out=y[:, :], in_=ysb)

    nc.compile()
    return nc


def _prep_weights(inputs):
    b0 = (np.asarray(inputs["bih0"]) + np.asarray(inputs["bhh0"])).astype(np.float32)
    b1 = (np.asarray(inputs["bih1"]) + np.asarray(inputs["bhh1"])).astype(np.float32)
    Wih0 = np.asarray(inputs["Wih0"], np.float32)
    Whh0 = np.asarray(inputs["Whh0"], np.float32)
    Wih1 = np.asarray(inputs["Wih1"], np.float32)
    Whh1 = np.asarray(inputs["Whh1"], np.float32)

    GI = {"i": 0, "f": 1, "g": 2, "o": 3}
    wall = np.zeros((123, 512), np.float32)
    for gate, gi in GI.items():
        rows = slice(gi * H, (gi + 1) * H)
        sc = 2.0 if gate == "g" else 1.0
        c = GBLK[gate] * 128
        # L0 -> output rows 0:50: recurrent h0 (K 0:50), x (K 114:122), bias
        wall[0:50, c:c + 50] = Whh0[rows, :].T * sc
        wall[114:122, c:c + 50] = Wih0[rows, :].T * sc
        wall[122, c:c + 50] = b0[rows] * sc
        # L1 -> output rows 64:114: input h0 (K 0:50), recurrent h1, bias
        wall[0:50, c + 64:c + 114] = Wih1[rows, :].T * sc
        wall[64:114, c + 64:c + 114] = Whh1[rows, :].T * sc
        wall[122, c + 64:c + 114] = b1[rows] * sc
    return wall.astype(np.float16)


def make_in_maps(inputs):
    x = np.asarray(inputs["x"], dtype=np.float32)
    wall = _prep_weights(inputs)
    wfin = np.zeros((128, 1), np.float16)
    wfin[64:114, 0] = np.asarray(inputs["Wlin"], dtype=np.float32)[0, :]

    in_maps = []
    for c in range(NCORES):
        xc = x[c * BC:(c + 1) * BC]              # [BC, T, D]
        xt = np.zeros((T, 14, BC), dtype=np.float16)
        xt[:, 0:D, :] = xc.transpose(1, 2, 0)
        xt[:, D, :] = 1.0                        # ones row (bias)
        in_maps.append({"xT": xt, "wall": wall, "wfin": wfin})
    return in_maps


def kernel(x, Wih0, Whh0, bih0, bhh0, Wih1, Whh1, bih1, bhh1, Wlin, blin):
    if "nc" not in _NC_CACHE:
        _NC_CACHE["nc"] = _build_nc()
    nc = _NC_CACHE["nc"]

    in_maps = make_in_maps(dict(x=x, Wih0=Wih0, Whh0=Whh0, bih0=bih0,
                                bhh0=bhh0, Wih1=Wih1, Whh1=Whh1, bih1=bih1,
                                bhh1=bhh1, Wlin=Wlin, blin=blin))
    res = run_bass_kernel_spmd(nc, in_maps, core_ids=list(range(NCORES)))
    out = np.empty((B, 1), dtype=np.float32)
    blin_v = np.float32(np.asarray(blin).reshape(-1)[0])
    for c in range(NCORES):
        out[c * BC:(c + 1) * BC, 0] = res.results[c]["y"][0] + blin_v
    return out
